# revision 1
# baseline (speedup 1.0000x reference)
"""Trainium2 Bass kernel for BatteryMoEFlattenIntraCycleMoELayer.

Computation (reference):
    gates = renorm(top2(softmax(logits) * mask))          # [B, E]
    x = cycle_curve_data.reshape(B, L, 900)
    out[b] = sum_e gates[b,e] * (x[b] @ W[e] + b[e])      # -> bf16 [B, L, 512]

Strategy:
  - Host: compute gates + top-2 routing (tiny), transpose x to feat-major
    [B, 901, 128] with a constant-1.0 row appended (folds the bias add into
    the matmul via weight augmentation W_aug = [W; b]).
  - Shard B across 8 cores (64 samples each). One SPMD program: routing is
    carried as *data* (per-sample W-slot element offsets, read into PE
    registers at runtime -> dynamic access patterns on the matmul moving
    operand), so the program does not depend on input values.
  - Device per sample: 2 experts x 8 K-chunks matmuls (N=512, float32r at
    full PE rate) accumulate x_aug @ W_aug[e] into 2 PSUM banks; ACT engine
    scales each by its gate (per-partition scalar AP from data); DVE adds
    and casts to bf16. A k-outer phase over the first 12 samples overlaps
    the 16.8 MB weight load with compute.
"""

import os
import sys

for _p in ("/opt/trn_rl_repo", "/root/.axon_site/_ro/trn_rl_repo"):
    if os.path.isdir(_p) and _p not in sys.path:
        sys.path.insert(0, _p)

import numpy as np
import ml_dtypes

import concourse.bass as bass
import concourse.mybir as mybir
import concourse.tile as tile
from concourse import bacc
from concourse.bass_utils import run_bass_kernel_spmd
from concourse.bass_values import RuntimeValue

B, L, CURVE_LEN = 512, 128, 300
FEAT = 3 * CURVE_LEN          # 900
FEAT_AUG = FEAT + 1           # 901 (bias row)
D_MODEL = 512
NUM_EXPERTS = 8
TOP_K = 2
EPS = 1e-9
N_CORES = 8
S = B // N_CORES              # 64 samples per core
N_KCH = 8                     # K chunks: 7 x 128 + 1 x 5
K_LAST = FEAT_AUG - 7 * 128   # 5

# matmul input dtypes: float32r streams fp32 bits at full PE rate (N>=256);
# bf16 variants halve DMA traffic at some precision cost (env-switchable for
# experiments; X_DT = stationary x dtype, W_DT = moving W dtype)
_DT_MAP = {"f32r": mybir.dt.float32r, "bf16": mybir.dt.bfloat16}
X_DT = _DT_MAP[os.environ.get("KERNEL_X_DT", "f32r")]
W_DT = _DT_MAP[os.environ.get("KERNEL_W_DT", "f32r")]

_CACHE = {}


def _build_nc():
    """Build the SPMD Bass program (routing-independent)."""
    nc = bacc.Bacc(trn_type="TRN2")
    f32 = mybir.dt.float32
    bf16 = mybir.dt.bfloat16
    i32 = mybir.dt.int32

    # x chunks 0..6: [S, 7*128, 128]; tail chunk (rows 896..900 + copy at
    # partition offset 32) as separate [S, 37, 128] tensor
    xt_h = nc.declare_dram_parameter("xt", [S, 128, 7 * 128], X_DT, isOutput=False)
    xtail_h = nc.declare_dram_parameter("xtail", [S, K_LAST, L], X_DT, isOutput=False)
    # w laid out per k-chunk: [k, part(<=128), expert, 512]
    w_h = nc.declare_dram_parameter("w", [N_KCH, 128, NUM_EXPERTS, D_MODEL], W_DT,
                                    isOutput=False)
    g_h = nc.declare_dram_parameter("g", [128, 2 * S], f32, isOutput=False)
    widx_h = nc.declare_dram_parameter("widx", [1, 2 * S], i32, isOutput=False)
    y_h = nc.declare_dram_parameter("y", [S, L, D_MODEL], bf16, isOutput=True)

    with tile.TileContext(nc) as tc:
        with (
            tc.tile_pool(name="cpool", bufs=1) as cpool,
            tc.tile_pool(name="xpool", bufs=12) as xpool,
            tc.tile_pool(name="tpool", bufs=4) as tpool,
            tc.tile_pool(name="opool", bufs=3) as opool,
            tc.tile_pool(name="pspool", bufs=8, space="PSUM") as pspool,
        ):
            # --- constants: gates, routing offsets, weights ---
            g_sb = cpool.tile([128, 2 * S], f32)
            nc.sync.dma_start(out=g_sb[:, :], in_=g_h[:, :])
            widx_sb = cpool.tile([1, 2 * S], i32)
            nc.sync.dma_start(out=widx_sb[:, :], in_=widx_h[:, :])

            # W tiles (DMAs issued after the phase-1 x preloads below)
            w_sb = []
            for k in range(N_KCH):
                wt = cpool.tile([128, NUM_EXPERTS * D_MODEL], W_DT,
                                name=f"w_sb_{k}")
                w_sb.append(wt)

            def load_w(k):
                # split each k-tile's DMA into 4 column chunks so the
                # transfers spread over many queues
                nsplit = 4
                WCOL = NUM_EXPERTS * D_MODEL // nsplit
                for c in range(nsplit):
                    nc.sync.dma_start(
                        out=w_sb[k][:, c * WCOL: (c + 1) * WCOL],
                        in_=w_h[k, :, :, :].rearrange("p e d -> p (e d)")[
                            :, c * WCOL: (c + 1) * WCOL
                        ],
                    )

            # ring of PE registers for the per-sample W-slot offsets;
            # loaded in batches of 8 (4 samples) to amortize TENSOR_LOAD cost
            NRING = 16
            wregs = [nc.tensor.alloc_register(f"widx_reg{i}") for i in range(NRING)]
            WMAX = (NUM_EXPERTS - 1) * D_MODEL

            def load_x(s):
                # host layout is partition-major: per partition one fully
                # contiguous 7*128*4B run; split in halves for 2-queue
                # parallelism; tail goes via SWDGE to keep HWDGE slots free
                x_sb = xpool.tile([128, N_KCH * 128], X_DT, tag="x",
                                  name=f"x_sb_{s}")
                H = 7 * 128 // 2   # 448
                nc.sync.dma_start(
                    out=x_sb[:, :H],
                    in_=xt_h[s, :, :H],
                )
                nc.sync.dma_start(
                    out=x_sb[:, H: 7 * 128],
                    in_=xt_h[s, :, H:],
                )
                nc.sync.dma_start(
                    out=x_sb[:K_LAST, 7 * 128: 7 * 128 + 128],
                    in_=xtail_h[s, :, :],
                )
                return x_sb

            def load_widx(s0):
                # 8 registers <- widx[2*s0 : 2*s0+8] (4 samples) in one load
                regs = [wregs[(2 * s0 + j) % NRING] for j in range(8)]
                nc.tensor.reg_load(regs, widx_sb[0:1, 2 * s0: 2 * s0 + 8])
                return [RuntimeValue(val=r, min_val=0, max_val=WMAX)
                        for r in regs]

            def mm_pair(ps, x_sb, rv, k, start, stop):
                kk = 128 if k < 7 else K_LAST
                nc.tensor.matmul(
                    ps[:, :], x_sb[:kk, k * 128: k * 128 + 128],
                    w_sb[k][:kk, bass.ds(rv, D_MODEL)],
                    start=start, stop=stop,
                )

            def combine(s, psA, psB):
                t1 = tpool.tile([128, D_MODEL], f32, tag="t", name=f"t1_{s}")
                t2 = tpool.tile([128, D_MODEL], f32, tag="t", name=f"t2_{s}")
                nc.scalar.mul(t1[:, :], psA[:, :], g_sb[:, 2 * s: 2 * s + 1])
                nc.scalar.mul(t2[:, :], psB[:, :], g_sb[:, 2 * s + 1: 2 * s + 2])
                o_sb = opool.tile([128, D_MODEL], bf16, tag="o", name=f"o_{s}")
                nc.vector.tensor_tensor(
                    o_sb[:, :], t1[:, :], t2[:, :], mybir.AluOpType.add
                )
                nc.sync.dma_start(out=y_h[s, :, :], in_=o_sb[:, :])

            # --- phase 1: first 12 samples, k-outer in 4-sample groups so
            # the PE starts as soon as w_sb[0] lands; DMA issue order is
            # interleaved to match consumption order ---
            PHASE1_GROUPS = 3
            p1_xs = [load_x(s) for s in range(4)]
            load_w(0)
            p1_xs += [load_x(4), load_x(5)]
            load_w(1)
            p1_xs += [load_x(6), load_x(7)]
            load_w(2)
            p1_xs += [load_x(8), load_x(9)]
            load_w(3)
            p1_xs += [load_x(10), load_x(11)]
            for k in range(4, N_KCH):
                load_w(k)
            for grp in range(PHASE1_GROUPS):
                s0 = grp * 4
                xs = p1_xs[s0: s0 + 4]
                rvs = load_widx(s0)
                pss = [pspool.tile([128, D_MODEL], f32, tag="ps",
                                   name=f"ps_{s0}_{j}") for j in range(8)]
                for k in range(N_KCH):
                    for j in range(4):
                        mm_pair(pss[2 * j], xs[j], rvs[2 * j], k,
                                start=(k == 0), stop=(k == N_KCH - 1))
                        mm_pair(pss[2 * j + 1], xs[j], rvs[2 * j + 1], k,
                                start=(k == 0), stop=(k == N_KCH - 1))
                for j in range(4):
                    combine(s0 + j, pss[2 * j], pss[2 * j + 1])

            # --- phase 2: steady state, sample-major ---
            for s in range(PHASE1_GROUPS * 4, S):
                x_sb = load_x(s)
                if s % 4 == 0:
                    _rvs = load_widx(s)
                    rv_cache = _rvs
                rvA = rv_cache[2 * (s % 4)]
                rvB = rv_cache[2 * (s % 4) + 1]

                psA = pspool.tile([128, D_MODEL], f32, tag="ps",
                                  name=f"psA_{s}")
                psB = pspool.tile([128, D_MODEL], f32, tag="ps",
                                  name=f"psB_{s}")
                for k in range(N_KCH):
                    mm_pair(psA, x_sb, rvA, k, start=(k == 0),
                            stop=(k == N_KCH - 1))
                    mm_pair(psB, x_sb, rvB, k, start=(k == 0),
                            stop=(k == N_KCH - 1))
                combine(s, psA, psB)

    nc.finalize()  # Bacc: reg graph-coloring + codegen passes, then freeze
    return nc


def _gates_np(logits, moe_masks):
    """Mirror reference _gates in numpy (fp32)."""
    lg = logits.astype(np.float32)
    m = lg.max(axis=1, keepdims=True)
    e = np.exp(lg - m)
    g = e / e.sum(axis=1, keepdims=True)
    g = g * (moe_masks == 1).astype(np.float32)
    # top-2, ties -> lower index first (matches jax.lax.top_k)
    top_idx = np.argsort(-g, axis=1, kind="stable")[:, :TOP_K]
    rows = np.arange(g.shape[0])[:, None]
    gsel = g[rows, top_idx]                                  # [B, 2]
    gsel = gsel / (gsel.sum(axis=1, keepdims=True) + EPS)
    return gsel.astype(np.float32), top_idx.astype(np.int32)


def _prep_inputs(cycle_curve_data, logits, moe_masks, W, b):
    gsel, top_idx = _gates_np(logits, moe_masks)

    xf = cycle_curve_data.reshape(B, L, FEAT).astype(np.float32, copy=False)
    # xt[s, p, k, l] = x[s, l, k*128 + p]  -> [B, 128, 7*128]
    xt = np.ascontiguousarray(
        xf[:, :, : 7 * 128].reshape(B, L, 7, 128).transpose(0, 3, 2, 1)
    ).reshape(B, 128, 7 * 128)
    xtail = np.empty((B, K_LAST, L), np.float32)
    xtail[:, :4, :] = xf[:, :, 7 * 128: FEAT].transpose(0, 2, 1)
    xtail[:, 4, :] = 1.0                                     # bias row

    w_aug = np.concatenate(
        [W.astype(np.float32), b.astype(np.float32)[:, None, :]], axis=1
    )                                                        # [E, 901, 512]
    w_host = np.zeros((N_KCH, 128, NUM_EXPERTS, D_MODEL), np.float32)
    for k in range(7):
        w_host[k] = w_aug[:, k * 128: (k + 1) * 128, :].transpose(1, 0, 2)
    w_host[7, :K_LAST] = w_aug[:, 7 * 128:, :].transpose(1, 0, 2)

    in_maps = []
    for c in range(N_CORES):
        sl = slice(c * S, (c + 1) * S)
        g_rep = np.broadcast_to(
            gsel[sl].reshape(1, 2 * S), (128, 2 * S)
        ).copy()
        widx = (top_idx[sl].reshape(1, 2 * S) * D_MODEL).astype(np.int32)
        x_np = np.float32 if os.environ.get("KERNEL_X_DT", "f32r") == "f32r" \
            else ml_dtypes.bfloat16
        w_np = np.float32 if os.environ.get("KERNEL_W_DT", "f32r") == "f32r" \
            else ml_dtypes.bfloat16
        in_maps.append({
            "xt": np.ascontiguousarray(xt[sl]).astype(x_np),
            "xtail": xtail[sl].astype(x_np),
            "w": w_host.astype(w_np),
            "g": g_rep,
            "widx": widx,
        })
    return in_maps


def _patch_ldw_opt():
    """Let walrus merge back-to-back LDWEIGHTS with identical stationary
    operands (the A/B expert matmuls share the x chunk): rewrite the
    hardcoded --enable-ldw-opt=false in the compile command."""
    from concourse import bass_utils as _bu
    if getattr(_bu, "_ldw_patched", False):
        return
    _orig = _bu.run_command

    def _rc(cmd, *a, **kw):
        cmd = [c.replace("--enable-ldw-opt=false", "--enable-ldw-opt=true")
               if isinstance(c, str) else c for c in cmd]
        return _orig(cmd, *a, **kw)

    _bu.run_command = _rc
    _bu._ldw_patched = True


def kernel(cycle_curve_data, logits, moe_masks, W, b):
    if os.environ.get("KERNEL_LDW_OPT", "0") == "1":
        _patch_ldw_opt()
    if "nc" not in _CACHE:
        _CACHE["nc"] = _build_nc()
    nc = _CACHE["nc"]

    in_maps = _prep_inputs(cycle_curve_data, logits, moe_masks, W, b)

    trace = bool(int(os.environ.get("KERNEL_PROFILE", "0")))
    res = run_bass_kernel_spmd(
        nc, in_maps, core_ids=list(range(N_CORES)), trace=trace
    )
    _CACHE["last_results"] = res

    out = np.empty((B, L, D_MODEL), ml_dtypes.bfloat16)
    for c in range(N_CORES):
        out[c * S: (c + 1) * S] = res.results[c]["y"]
    return out



# revision 4
# speedup vs baseline: 1.1556x; 1.1556x over previous
"""Trainium2 Bass kernel for BatteryMoEFlattenIntraCycleMoELayer.

Computation (reference):
    gates = renorm(top2(softmax(logits) * mask))          # [B, E]
    x = cycle_curve_data.reshape(B, L, 900)
    out[b] = sum_e gates[b,e] * (x[b] @ W[e] + b[e])      # -> bf16 [B, L, 512]

Strategy:
  - Host: compute gates + top-2 routing (tiny), transpose x to feat-major
    [B, 901, 128] with a constant-1.0 row appended (folds the bias add into
    the matmul via weight augmentation W_aug = [W; b]).
  - Shard B across 8 cores (64 samples each). One SPMD program: routing is
    carried as *data* (per-sample W-slot element offsets, read into PE
    registers at runtime -> dynamic access patterns on the matmul moving
    operand), so the program does not depend on input values.
  - Device per sample: 2 experts x 8 K-chunks matmuls (N=512, float32r at
    full PE rate) accumulate x_aug @ W_aug[e] into 2 PSUM banks; ACT engine
    scales each by its gate (per-partition scalar AP from data); DVE adds
    and casts to bf16. A k-outer phase over the first 12 samples overlaps
    the 16.8 MB weight load with compute.
"""

import os
import sys

for _p in ("/opt/trn_rl_repo", "/root/.axon_site/_ro/trn_rl_repo"):
    if os.path.isdir(_p) and _p not in sys.path:
        sys.path.insert(0, _p)

import numpy as np
import ml_dtypes

import concourse.bass as bass
import concourse.mybir as mybir
import concourse.tile as tile
from concourse import bacc
from concourse.bass_utils import run_bass_kernel_spmd
from concourse.bass_values import RuntimeValue

B, L, CURVE_LEN = 512, 128, 300
FEAT = 3 * CURVE_LEN          # 900
FEAT_AUG = FEAT + 1           # 901 (bias row)
D_MODEL = 512
NUM_EXPERTS = 8
TOP_K = 2
EPS = 1e-9
N_CORES = 8
S = B // N_CORES              # 64 samples per core
N_KCH = 8                     # K chunks: 7 x 128 + 1 x 5
K_LAST = FEAT_AUG - 7 * 128   # 5

# matmul input dtypes: float32r streams fp32 bits at full PE rate (N>=256);
# bf16 variants halve DMA traffic at some precision cost (env-switchable for
# experiments; X_DT = stationary x dtype, W_DT = moving W dtype)
_DT_MAP = {"f32r": mybir.dt.float32r, "bf16": mybir.dt.bfloat16}
X_DT = _DT_MAP[os.environ.get("KERNEL_X_DT", "f32r")]
W_DT = _DT_MAP[os.environ.get("KERNEL_W_DT", "f32r")]

_CACHE = {}


def _build_nc():
    """Build the SPMD Bass program (routing-independent)."""
    nc = bacc.Bacc(trn_type="TRN2")
    f32 = mybir.dt.float32
    bf16 = mybir.dt.bfloat16
    i32 = mybir.dt.int32

    # x chunks 0..6: [S, 7*128, 128]; tail chunk (rows 896..900 + copy at
    # partition offset 32) as separate [S, 37, 128] tensor
    xt_h = nc.declare_dram_parameter("xt", [S, 128, 7 * 128], X_DT, isOutput=False)
    xtail_h = nc.declare_dram_parameter("xtail", [S, K_LAST, L], X_DT, isOutput=False)
    # w laid out per k-chunk: [k, part(<=128), expert, 512]
    w_h = nc.declare_dram_parameter("w", [N_KCH, 128, NUM_EXPERTS, D_MODEL], W_DT,
                                    isOutput=False)
    g_h = nc.declare_dram_parameter("g", [128, 2 * S], f32, isOutput=False)
    widx_h = nc.declare_dram_parameter("widx", [1, 2 * S], i32, isOutput=False)
    y_h = nc.declare_dram_parameter("y", [S, L, D_MODEL], bf16, isOutput=True)

    with tile.TileContext(nc) as tc:
        with (
            tc.tile_pool(name="cpool", bufs=1) as cpool,
            tc.tile_pool(name="xpool", bufs=12) as xpool,
            tc.tile_pool(name="tpool", bufs=4) as tpool,
            tc.tile_pool(name="opool", bufs=3) as opool,
            tc.tile_pool(name="pspool", bufs=8, space="PSUM") as pspool,
        ):
            # --- constants: gates, routing offsets, weights ---
            g_sb = cpool.tile([128, 2 * S], f32)
            nc.sync.dma_start(out=g_sb[:, :], in_=g_h[:, :])
            widx_sb = cpool.tile([1, 2 * S], i32)
            nc.sync.dma_start(out=widx_sb[:, :], in_=widx_h[:, :])

            # W tiles (DMAs issued after the phase-1 x preloads below)
            w_sb = []
            for k in range(N_KCH):
                wt = cpool.tile([128, NUM_EXPERTS * D_MODEL], W_DT,
                                name=f"w_sb_{k}")
                w_sb.append(wt)

            def load_w(k):
                # split each k-tile's DMA into 4 column chunks so the
                # transfers spread over many queues
                nsplit = 4
                WCOL = NUM_EXPERTS * D_MODEL // nsplit
                for c in range(nsplit):
                    nc.sync.dma_start(
                        out=w_sb[k][:, c * WCOL: (c + 1) * WCOL],
                        in_=w_h[k, :, :, :].rearrange("p e d -> p (e d)")[
                            :, c * WCOL: (c + 1) * WCOL
                        ],
                    )

            # ring of PE registers for the per-sample W-slot offsets;
            # loaded in batches of 8 (4 samples) to amortize TENSOR_LOAD cost
            NRING = 16
            wregs = [nc.tensor.alloc_register(f"widx_reg{i}") for i in range(NRING)]
            WMAX = (NUM_EXPERTS - 1) * D_MODEL

            def load_x(s):
                # host layout is partition-major: per partition one fully
                # contiguous 7*128*4B run; split in halves for 2-queue
                # parallelism; tail goes via SWDGE to keep HWDGE slots free
                x_sb = xpool.tile([128, N_KCH * 128], X_DT, tag="x",
                                  name=f"x_sb_{s}")
                H = 7 * 128 // 2   # 448
                nc.sync.dma_start(
                    out=x_sb[:, :H],
                    in_=xt_h[s, :, :H],
                )
                nc.sync.dma_start(
                    out=x_sb[:, H: 7 * 128],
                    in_=xt_h[s, :, H:],
                )
                nc.sync.dma_start(
                    out=x_sb[:K_LAST, 7 * 128: 7 * 128 + 128],
                    in_=xtail_h[s, :, :],
                )
                return x_sb

            def load_widx(s0):
                # 8 registers <- widx[2*s0 : 2*s0+8] (4 samples) in one load
                regs = [wregs[(2 * s0 + j) % NRING] for j in range(8)]
                nc.tensor.reg_load(regs, widx_sb[0:1, 2 * s0: 2 * s0 + 8])
                return [RuntimeValue(val=r, min_val=0, max_val=WMAX)
                        for r in regs]

            def mm_pair(ps, x_sb, rv, k, start, stop):
                kk = 128 if k < 7 else K_LAST
                nc.tensor.matmul(
                    ps[:, :], x_sb[:kk, k * 128: k * 128 + 128],
                    w_sb[k][:kk, bass.ds(rv, D_MODEL)],
                    start=start, stop=stop,
                )

            def combine(s, psA, psB):
                t1 = tpool.tile([128, D_MODEL], f32, tag="t", name=f"t1_{s}")
                t2 = tpool.tile([128, D_MODEL], f32, tag="t", name=f"t2_{s}")
                nc.scalar.mul(t1[:, :], psA[:, :], g_sb[:, 2 * s: 2 * s + 1])
                nc.scalar.mul(t2[:, :], psB[:, :], g_sb[:, 2 * s + 1: 2 * s + 2])
                o_sb = opool.tile([128, D_MODEL], bf16, tag="o", name=f"o_{s}")
                nc.vector.tensor_tensor(
                    o_sb[:, :], t1[:, :], t2[:, :], mybir.AluOpType.add
                )
                nc.sync.dma_start(out=y_h[s, :, :], in_=o_sb[:, :])

            # --- phase 1: first 12 samples, k-outer in 4-sample groups so
            # the PE starts as soon as w_sb[0] lands; DMA issue order is
            # interleaved to match consumption order ---
            PHASE1_GROUPS = 3
            p1_xs = [load_x(s) for s in range(4)]
            load_w(0)
            p1_xs += [load_x(4), load_x(5)]
            load_w(1)
            p1_xs += [load_x(6), load_x(7)]
            load_w(2)
            p1_xs += [load_x(8), load_x(9)]
            load_w(3)
            p1_xs += [load_x(10), load_x(11)]
            for k in range(4, N_KCH):
                load_w(k)
            for grp in range(PHASE1_GROUPS):
                s0 = grp * 4
                xs = p1_xs[s0: s0 + 4]
                rvs = load_widx(s0)
                pss = [pspool.tile([128, D_MODEL], f32, tag="ps",
                                   name=f"ps_{s0}_{j}") for j in range(8)]
                for k in range(N_KCH):
                    for j in range(4):
                        mm_pair(pss[2 * j], xs[j], rvs[2 * j], k,
                                start=(k == 0), stop=(k == N_KCH - 1))
                        mm_pair(pss[2 * j + 1], xs[j], rvs[2 * j + 1], k,
                                start=(k == 0), stop=(k == N_KCH - 1))
                for j in range(4):
                    combine(s0 + j, pss[2 * j], pss[2 * j + 1])

            # --- phase 2: steady state, sample-major ---
            for s in range(PHASE1_GROUPS * 4, S):
                x_sb = load_x(s)
                if s % 4 == 0:
                    _rvs = load_widx(s)
                    rv_cache = _rvs
                rvA = rv_cache[2 * (s % 4)]
                rvB = rv_cache[2 * (s % 4) + 1]

                psA = pspool.tile([128, D_MODEL], f32, tag="ps",
                                  name=f"psA_{s}")
                psB = pspool.tile([128, D_MODEL], f32, tag="ps",
                                  name=f"psB_{s}")
                for k in range(N_KCH):
                    mm_pair(psA, x_sb, rvA, k, start=(k == 0),
                            stop=(k == N_KCH - 1))
                    mm_pair(psB, x_sb, rvB, k, start=(k == 0),
                            stop=(k == N_KCH - 1))
                combine(s, psA, psB)

    nc.finalize()  # Bacc: reg graph-coloring + codegen passes, then freeze
    return nc


def _gates_np(logits, moe_masks):
    """Mirror reference _gates in numpy (fp32)."""
    lg = logits.astype(np.float32)
    m = lg.max(axis=1, keepdims=True)
    e = np.exp(lg - m)
    g = e / e.sum(axis=1, keepdims=True)
    g = g * (moe_masks == 1).astype(np.float32)
    # top-2, ties -> lower index first (matches jax.lax.top_k)
    top_idx = np.argsort(-g, axis=1, kind="stable")[:, :TOP_K]
    rows = np.arange(g.shape[0])[:, None]
    gsel = g[rows, top_idx]                                  # [B, 2]
    gsel = gsel / (gsel.sum(axis=1, keepdims=True) + EPS)
    return gsel.astype(np.float32), top_idx.astype(np.int32)


def _prep_inputs(cycle_curve_data, logits, moe_masks, W, b):
    gsel, top_idx = _gates_np(logits, moe_masks)

    xf = cycle_curve_data.reshape(B, L, FEAT).astype(np.float32, copy=False)
    # xt[s, p, k, l] = x[s, l, k*128 + p]  -> [B, 128, 7*128]
    xt = np.ascontiguousarray(
        xf[:, :, : 7 * 128].reshape(B, L, 7, 128).transpose(0, 3, 2, 1)
    ).reshape(B, 128, 7 * 128)
    xtail = np.empty((B, K_LAST, L), np.float32)
    xtail[:, :4, :] = xf[:, :, 7 * 128: FEAT].transpose(0, 2, 1)
    xtail[:, 4, :] = 1.0                                     # bias row

    w_aug = np.concatenate(
        [W.astype(np.float32), b.astype(np.float32)[:, None, :]], axis=1
    )                                                        # [E, 901, 512]
    w_host = np.zeros((N_KCH, 128, NUM_EXPERTS, D_MODEL), np.float32)
    for k in range(7):
        w_host[k] = w_aug[:, k * 128: (k + 1) * 128, :].transpose(1, 0, 2)
    w_host[7, :K_LAST] = w_aug[:, 7 * 128:, :].transpose(1, 0, 2)

    in_maps = []
    for c in range(N_CORES):
        sl = slice(c * S, (c + 1) * S)
        g_rep = np.broadcast_to(
            gsel[sl].reshape(1, 2 * S), (128, 2 * S)
        ).copy()
        widx = (top_idx[sl].reshape(1, 2 * S) * D_MODEL).astype(np.int32)
        x_np = np.float32 if os.environ.get("KERNEL_X_DT", "f32r") == "f32r" \
            else ml_dtypes.bfloat16
        w_np = np.float32 if os.environ.get("KERNEL_W_DT", "f32r") == "f32r" \
            else ml_dtypes.bfloat16
        in_maps.append({
            "xt": np.ascontiguousarray(xt[sl]).astype(x_np),
            "xtail": xtail[sl].astype(x_np),
            "w": w_host.astype(w_np),
            "g": g_rep,
            "widx": widx,
        })
    return in_maps


def _patch_ldw_opt():
    """Let walrus merge back-to-back LDWEIGHTS with identical stationary
    operands (the A/B expert matmuls share the x chunk): rewrite the
    hardcoded --enable-ldw-opt=false in the compile command."""
    from concourse import bass_utils as _bu
    if getattr(_bu, "_ldw_patched", False):
        return
    _orig = _bu.run_command

    def _rc(cmd, *a, **kw):
        cmd = [c.replace("--enable-ldw-opt=false", "--enable-ldw-opt=true")
               if isinstance(c, str) else c for c in cmd]
        return _orig(cmd, *a, **kw)

    _bu.run_command = _rc
    _bu._ldw_patched = True


def kernel(cycle_curve_data, logits, moe_masks, W, b):
    if os.environ.get("KERNEL_LDW_OPT", "0") == "1":
        _patch_ldw_opt()
    if "nc" not in _CACHE:
        _CACHE["nc"] = _build_nc()
    nc = _CACHE["nc"]

    in_maps = _prep_inputs(cycle_curve_data, logits, moe_masks, W, b)

    trace = bool(int(os.environ.get("KERNEL_PROFILE", "0")))
    res = run_bass_kernel_spmd(
        nc, in_maps, core_ids=list(range(N_CORES)), trace=trace
    )
    _CACHE["last_results"] = res

    out = np.empty((B, L, D_MODEL), ml_dtypes.bfloat16)
    for c in range(N_CORES):
        out[c * S: (c + 1) * S] = res.results[c]["y"]
    return out



# revision 5
# speedup vs baseline: 1.2191x; 1.0549x over previous
"""Trainium2 Bass kernel for BatteryMoEFlattenIntraCycleMoELayer.

Computation (reference):
    gates = renorm(top2(softmax(logits) * mask))          # [B, E]
    x = cycle_curve_data.reshape(B, L, 900)
    out[b] = sum_e gates[b,e] * (x[b] @ W[e] + b[e])      # -> bf16 [B, L, 512]

Strategy (bf16, K padded to 1024):
  - Host: compute gates + top-2 routing (tiny), append the bias row
    (K=901) and zero-pad K to 1024; pack x feat-major [B, 128, 8, 128]
    (k = sub*128 + p) in bf16.  W augmented/padded the same way:
    [8, 128, E, 512] per k-chunk, bf16.
  - Shard B across 8 cores (64 samples each).  One SPMD program:
    routing carried as data (per-sample W-slot element offsets read
    into PE registers -> dynamic access patterns on the matmul moving
    operand), so the program does not depend on input values.
  - Device per sample: 2 experts x 8 uniform K-chunk matmuls
    ([128,128]x[128,512] bf16, ~239 ns each) accumulate into 2 PSUM
    banks; ACT scales each by its gate; DVE adds and casts to bf16.
  - Phase 1 runs the first 12 samples k-outer in 4-sample groups so
    the PE starts as soon as w_sb[0] lands; the startup DMA order
    prioritizes w0 + the first group's x tiles.
"""

import os
import sys

for _p in ("/opt/trn_rl_repo", "/root/.axon_site/_ro/trn_rl_repo"):
    if os.path.isdir(_p) and _p not in sys.path:
        sys.path.insert(0, _p)

import numpy as np
import ml_dtypes

import concourse.bass as bass
import concourse.mybir as mybir
import concourse.tile as tile
from concourse import bacc
from concourse.bass_utils import run_bass_kernel_spmd
from concourse.bass_values import RuntimeValue

B, L, CURVE_LEN = 512, 128, 300
FEAT = 3 * CURVE_LEN          # 900
FEAT_AUG = FEAT + 1           # 901 (bias row)
K_PAD = 1024                  # zero-padded K: 8 uniform chunks of 128
N_KCH = 8
D_MODEL = 512
NUM_EXPERTS = 8
TOP_K = 2
EPS = 1e-9
N_CORES = 8
S = B // N_CORES              # 64 samples per core

BF16 = ml_dtypes.bfloat16

_CACHE = {}


def _build_nc():
    """Build the SPMD Bass program (routing-independent)."""
    nc = bacc.Bacc(trn_type="TRN2")
    f32 = mybir.dt.float32
    bf16 = mybir.dt.bfloat16
    i32 = mybir.dt.int32

    # x: [S, part, sub, L] bf16 with k = sub*128 + part (zero-padded)
    xt_h = nc.declare_dram_parameter("xt", [S, 128, N_KCH, L], bf16,
                                     isOutput=False)
    # w per k-chunk: [k, part, expert*512] bf16 (zero-padded rows)
    w_h = nc.declare_dram_parameter(
        "w", [N_KCH, 128, NUM_EXPERTS * D_MODEL], bf16, isOutput=False)
    g_h = nc.declare_dram_parameter("g", [128, 2 * S], f32, isOutput=False)
    widx_h = nc.declare_dram_parameter("widx", [1, 2 * S], i32, isOutput=False)
    y_h = nc.declare_dram_parameter("y", [S, L, D_MODEL], bf16, isOutput=True)

    with tile.TileContext(nc) as tc:
        with (
            tc.tile_pool(name="cpool", bufs=1) as cpool,
            tc.tile_pool(name="xpool", bufs=12) as xpool,
            tc.tile_pool(name="tpool", bufs=4) as tpool,
            tc.tile_pool(name="opool", bufs=3) as opool,
            tc.tile_pool(name="pspool", bufs=8, space="PSUM") as pspool,
        ):
            # --- constants: gates, routing offsets, weights ---
            g_sb = cpool.tile([128, 2 * S], f32)
            nc.sync.dma_start(out=g_sb[:, :], in_=g_h[:, :])
            widx_sb = cpool.tile([1, 2 * S], i32)
            nc.sync.dma_start(out=widx_sb[:, :], in_=widx_h[:, :])

            w_sb = []
            for k in range(N_KCH):
                wt = cpool.tile([128, NUM_EXPERTS * D_MODEL], bf16,
                                name=f"w_sb_{k}")
                w_sb.append(wt)

            def load_w(k):
                # 4 column chunks per k-tile to spread across queues
                nsplit = 4
                WCOL = NUM_EXPERTS * D_MODEL // nsplit
                for c in range(nsplit):
                    nc.sync.dma_start(
                        out=w_sb[k][:, c * WCOL: (c + 1) * WCOL],
                        in_=w_h[k, :, c * WCOL: (c + 1) * WCOL],
                    )

            # ring of PE registers for the per-sample W-slot offsets
            NRING = 16
            wregs = [nc.tensor.alloc_register(f"widx_reg{i}") for i in range(NRING)]
            WMAX = (NUM_EXPERTS - 1) * D_MODEL

            def load_x(s):
                # per-partition contiguous 2 KB run; split in halves for
                # 2-queue parallelism
                x_sb = xpool.tile([128, N_KCH, L], bf16, tag="x",
                                  name=f"x_sb_{s}")
                nc.sync.dma_start(out=x_sb[:, :4, :], in_=xt_h[s, :, :4, :])
                nc.sync.dma_start(out=x_sb[:, 4:, :], in_=xt_h[s, :, 4:, :])
                return x_sb

            def load_widx(s0):
                # 8 registers <- widx[2*s0 : 2*s0+8] (4 samples) in one load
                regs = [wregs[(2 * s0 + j) % NRING] for j in range(8)]
                nc.tensor.reg_load(regs, widx_sb[0:1, 2 * s0: 2 * s0 + 8])
                return [RuntimeValue(val=r, min_val=0, max_val=WMAX)
                        for r in regs]

            def mm_pair(ps, x_sb, rv, k, start, stop):
                nc.tensor.matmul(
                    ps[:, :], x_sb[:, k, :],
                    w_sb[k][:, bass.ds(rv, D_MODEL)],
                    start=start, stop=stop,
                )

            def combine(s, psA, psB):
                t1 = tpool.tile([128, D_MODEL], f32, tag="t", name=f"t1_{s}")
                t2 = tpool.tile([128, D_MODEL], f32, tag="t", name=f"t2_{s}")
                nc.scalar.mul(t1[:, :], psA[:, :], g_sb[:, 2 * s: 2 * s + 1])
                nc.scalar.mul(t2[:, :], psB[:, :], g_sb[:, 2 * s + 1: 2 * s + 2])
                o_sb = opool.tile([128, D_MODEL], bf16, tag="o", name=f"o_{s}")
                nc.vector.tensor_tensor(
                    o_sb[:, :], t1[:, :], t2[:, :], mybir.AluOpType.add
                )
                nc.sync.dma_start(out=y_h[s, :, :], in_=o_sb[:, :])

            # --- startup DMA order: first k-wave deps first ---
            PHASE1_GROUPS = 3
            load_w(0)
            p1_xs = [load_x(s) for s in range(4)]
            load_w(1)
            load_w(2)
            p1_xs += [load_x(4), load_x(5)]
            load_w(3)
            p1_xs += [load_x(6), load_x(7)]
            load_w(4)
            p1_xs += [load_x(8), load_x(9)]
            load_w(5)
            p1_xs += [load_x(10), load_x(11)]
            for k in range(6, N_KCH):
                load_w(k)

            # --- phase 1: first 12 samples, k-outer in 4-sample groups ---
            for grp in range(PHASE1_GROUPS):
                s0 = grp * 4
                xs = p1_xs[s0: s0 + 4]
                rvs = load_widx(s0)
                pss = [pspool.tile([128, D_MODEL], f32, tag="ps",
                                   name=f"ps_{s0}_{j}") for j in range(8)]
                for k in range(N_KCH):
                    for j in range(4):
                        mm_pair(pss[2 * j], xs[j], rvs[2 * j], k,
                                start=(k == 0), stop=(k == N_KCH - 1))
                        mm_pair(pss[2 * j + 1], xs[j], rvs[2 * j + 1], k,
                                start=(k == 0), stop=(k == N_KCH - 1))
                for j in range(4):
                    combine(s0 + j, pss[2 * j], pss[2 * j + 1])

            # --- phase 2: steady state, sample-major ---
            for s in range(PHASE1_GROUPS * 4, S):
                x_sb = load_x(s)
                if s % 4 == 0:
                    _rvs = load_widx(s)
                    rv_cache = _rvs
                rvA = rv_cache[2 * (s % 4)]
                rvB = rv_cache[2 * (s % 4) + 1]

                psA = pspool.tile([128, D_MODEL], f32, tag="ps",
                                  name=f"psA_{s}")
                psB = pspool.tile([128, D_MODEL], f32, tag="ps",
                                  name=f"psB_{s}")
                for k in range(N_KCH):
                    mm_pair(psA, x_sb, rvA, k, start=(k == 0),
                            stop=(k == N_KCH - 1))
                    mm_pair(psB, x_sb, rvB, k, start=(k == 0),
                            stop=(k == N_KCH - 1))
                combine(s, psA, psB)

    nc.finalize()  # Bacc: reg graph-coloring + codegen passes, then freeze
    return nc


def _gates_np(logits, moe_masks):
    """Mirror reference _gates in numpy (fp32)."""
    lg = logits.astype(np.float32)
    m = lg.max(axis=1, keepdims=True)
    e = np.exp(lg - m)
    g = e / e.sum(axis=1, keepdims=True)
    g = g * (moe_masks == 1).astype(np.float32)
    # top-2, ties -> lower index first (matches jax.lax.top_k)
    top_idx = np.argsort(-g, axis=1, kind="stable")[:, :TOP_K]
    rows = np.arange(g.shape[0])[:, None]
    gsel = g[rows, top_idx]                                  # [B, 2]
    gsel = gsel / (gsel.sum(axis=1, keepdims=True) + EPS)
    return gsel.astype(np.float32), top_idx.astype(np.int32)


def _prep_inputs(cycle_curve_data, logits, moe_masks, W, b):
    gsel, top_idx = _gates_np(logits, moe_masks)

    xf = cycle_curve_data.reshape(B, L, FEAT).astype(np.float32, copy=False)
    # pad K to 1024 with the bias-ones row at k=900; pack [B, p, sub, L]
    xq = np.zeros((B, L, K_PAD), BF16)
    xq[:, :, :FEAT] = xf.astype(BF16)
    xq[:, :, FEAT] = np.float32(1.0)
    xt = np.ascontiguousarray(
        xq.reshape(B, L, N_KCH, 128).transpose(0, 3, 2, 1))

    w_aug = np.zeros((NUM_EXPERTS, K_PAD, D_MODEL), np.float32)
    w_aug[:, :FEAT, :] = W.astype(np.float32)
    w_aug[:, FEAT, :] = b.astype(np.float32)
    # [E, k, p, 512] -> [k, p, E, 512]
    w_host = np.ascontiguousarray(
        w_aug.astype(BF16).reshape(NUM_EXPERTS, N_KCH, 128, D_MODEL)
        .transpose(1, 2, 0, 3)).reshape(N_KCH, 128, NUM_EXPERTS * D_MODEL)

    in_maps = []
    for c in range(N_CORES):
        sl = slice(c * S, (c + 1) * S)
        g_rep = np.broadcast_to(
            gsel[sl].reshape(1, 2 * S), (128, 2 * S)
        ).copy()
        widx = (top_idx[sl].reshape(1, 2 * S) * D_MODEL).astype(np.int32)
        in_maps.append({
            "xt": xt[sl],
            "w": w_host,
            "g": g_rep,
            "widx": widx,
        })
    return in_maps


def _patch_ldw_opt():
    """Let walrus merge back-to-back LDWEIGHTS with identical stationary
    operands (the A/B expert matmuls share the x chunk): rewrite the
    hardcoded --enable-ldw-opt=false in the compile command."""
    from concourse import bass_utils as _bu
    if getattr(_bu, "_ldw_patched", False):
        return
    _orig = _bu.run_command

    def _rc(cmd, *a, **kw):
        cmd = [c.replace("--enable-ldw-opt=false", "--enable-ldw-opt=true")
               if isinstance(c, str) else c for c in cmd]
        return _orig(cmd, *a, **kw)

    _bu.run_command = _rc
    _bu._ldw_patched = True


def kernel(cycle_curve_data, logits, moe_masks, W, b):
    if os.environ.get("KERNEL_LDW_OPT", "0") == "1":
        _patch_ldw_opt()
    if "nc" not in _CACHE:
        _CACHE["nc"] = _build_nc()
    nc = _CACHE["nc"]

    in_maps = _prep_inputs(cycle_curve_data, logits, moe_masks, W, b)

    trace = bool(int(os.environ.get("KERNEL_PROFILE", "0")))
    res = run_bass_kernel_spmd(
        nc, in_maps, core_ids=list(range(N_CORES)), trace=trace
    )
    _CACHE["last_results"] = res

    out = np.empty((B, L, D_MODEL), ml_dtypes.bfloat16)
    for c in range(N_CORES):
        out[c * S: (c + 1) * S] = res.results[c]["y"]
    return out


# revision 7
# speedup vs baseline: 1.2804x; 1.0503x over previous
"""Trainium2 Bass kernel for BatteryMoEFlattenIntraCycleMoELayer.

Computation (reference):
    gates = renorm(top2(softmax(logits) * mask))          # [B, E]
    x = cycle_curve_data.reshape(B, L, 900)
    out[b] = sum_e gates[b,e] * (x[b] @ W[e] + b[e])      # -> bf16 [B, L, 512]

Strategy (bf16, gate-prescaled x, K padded to 1024):
  - Host: compute gates + top-2 routing; build TWO gate-prescaled
    copies of x per sample (xA = gA*x_aug, xB = gB*x_aug, bias row
    included), packed feat-major [B, 128, 8, 128] bf16 (k = sub*128+p,
    zero-padded K 901->1024).  W augmented/padded the same way.
  - Because x carries the gate, both experts' matmuls accumulate into
    ONE PSUM bank per sample: 16 uniform [128,128]x[128,512] bf16
    matmuls -> psum; the combine collapses to a single ACT-engine
    copy/cast psum -> bf16.  One bank/sample makes 8 samples
    k-in-flight possible, which hides the 8.4 MB weight stream during
    phase 1 (k-outer waves of 16 matmuls >= per-tile DMA time).
  - Shard B across 8 cores (64 samples each); routing carried as data
    (per-sample W-slot offsets read into PE registers -> dynamic APs
    on the moving W operand), so one SPMD program serves all cores.
"""

import os
import sys

for _p in ("/opt/trn_rl_repo", "/root/.axon_site/_ro/trn_rl_repo"):
    if os.path.isdir(_p) and _p not in sys.path:
        sys.path.insert(0, _p)

import numpy as np
import ml_dtypes

import concourse.bass as bass
import concourse.mybir as mybir
import concourse.tile as tile
from concourse import bacc
from concourse.bass_utils import run_bass_kernel_spmd
from concourse.bass_values import RuntimeValue

B, L, CURVE_LEN = 512, 128, 300
FEAT = 3 * CURVE_LEN          # 900
FEAT_AUG = FEAT + 1           # 901 (bias row)
K_PAD = 1024                  # zero-padded K: 8 uniform chunks of 128
N_KCH = 8
D_MODEL = 512
NUM_EXPERTS = 8
TOP_K = 2
EPS = 1e-9
N_CORES = 8
S = B // N_CORES              # 64 samples per core

BF16 = ml_dtypes.bfloat16

_CACHE = {}


def _build_nc():
    """Build the SPMD Bass program (routing-independent)."""
    nc = bacc.Bacc(trn_type="TRN2")
    f32 = mybir.dt.float32
    bf16 = mybir.dt.bfloat16
    i32 = mybir.dt.int32

    # gate-prescaled x copies: [S, part, sub, L] bf16, k = sub*128 + part
    xa_h = nc.declare_dram_parameter("xa", [S, 128, N_KCH, L], bf16,
                                     isOutput=False)
    xb_h = nc.declare_dram_parameter("xb", [S, 128, N_KCH, L], bf16,
                                     isOutput=False)
    # w per k-chunk: [k, part, expert*512] bf16 (zero-padded rows)
    w_h = nc.declare_dram_parameter(
        "w", [N_KCH, 128, NUM_EXPERTS * D_MODEL], bf16, isOutput=False)
    widx_h = nc.declare_dram_parameter("widx", [1, 2 * S], i32, isOutput=False)
    y_h = nc.declare_dram_parameter("y", [S, L, D_MODEL], bf16, isOutput=True)

    with tile.TileContext(nc) as tc:
        with (
            tc.tile_pool(name="cpool", bufs=1) as cpool,
            tc.tile_pool(name="xpool", bufs=20) as xpool,
            tc.tile_pool(name="opool", bufs=6) as opool,
            tc.tile_pool(name="pspool", bufs=8, space="PSUM") as pspool,
        ):
            widx_sb = cpool.tile([1, 2 * S], i32)
            nc.sync.dma_start(out=widx_sb[:, :], in_=widx_h[:, :])

            w_sb = []
            for k in range(N_KCH):
                wt = cpool.tile([128, NUM_EXPERTS * D_MODEL], bf16,
                                name=f"w_sb_{k}")
                w_sb.append(wt)

            def load_w(k):
                # 4 column chunks per k-tile to spread across queues
                nsplit = 4
                WCOL = NUM_EXPERTS * D_MODEL // nsplit
                for c in range(nsplit):
                    nc.sync.dma_start(
                        out=w_sb[k][:, c * WCOL: (c + 1) * WCOL],
                        in_=w_h[k, :, c * WCOL: (c + 1) * WCOL],
                    )

            # ring of PE registers for the per-sample W-slot offsets
            NRING = 16
            wregs = [nc.tensor.alloc_register(f"widx_reg{i}")
                     for i in range(NRING)]
            WMAX = (NUM_EXPERTS - 1) * D_MODEL

            def load_x(s):
                xA = xpool.tile([128, N_KCH, L], bf16, tag="x",
                                name=f"xa_sb_{s}")
                xB = xpool.tile([128, N_KCH, L], bf16, tag="x",
                                name=f"xb_sb_{s}")
                nc.sync.dma_start(out=xA[:, :, :], in_=xa_h[s, :, :, :])
                nc.sync.dma_start(out=xB[:, :, :], in_=xb_h[s, :, :, :])
                return xA, xB

            def load_widx(s0):
                # 8 registers <- widx[2*s0 : 2*s0+8] (4 samples) in one load
                regs = [wregs[(2 * s0 + j) % NRING] for j in range(8)]
                nc.tensor.reg_load(regs, widx_sb[0:1, 2 * s0: 2 * s0 + 8])
                return [RuntimeValue(val=r, min_val=0, max_val=WMAX)
                        for r in regs]

            def mm(ps, x_sb, rv, k, start, stop):
                nc.tensor.matmul(
                    ps[:, :], x_sb[:, k, :],
                    w_sb[k][:, bass.ds(rv, D_MODEL)],
                    start=start, stop=stop,
                )

            def combine(s, ps):
                o_sb = opool.tile([128, D_MODEL], bf16, tag="o", name=f"o_{s}")
                nc.scalar.copy(o_sb[:, :], ps[:, :])
                nc.sync.dma_start(out=y_h[s, :, :], in_=o_sb[:, :])

            def kouter_group(samples, xs, rv_of):
                """k-outer over a group of samples, 1 PSUM bank each."""
                pss = {s: pspool.tile([128, D_MODEL], f32, tag="ps",
                                      name=f"ps_{s}") for s in samples}
                for k in range(N_KCH):
                    for s in samples:
                        rvA, rvB = rv_of[s]
                        mm(pss[s], xs[s][0], rvA, k,
                           start=(k == 0), stop=False)
                        mm(pss[s], xs[s][1], rvB, k,
                           start=False, stop=(k == N_KCH - 1))
                for s in samples:
                    combine(s, pss[s])

            # --- startup DMA order: first group's deps first, W stream
            # interleaved with later groups' x tiles ---
            xs = {}
            load_w(0)
            for s in range(0, 4):
                xs[s] = load_x(s)
            load_w(1)
            for s in range(4, 8):
                xs[s] = load_x(s)
            load_w(2)
            for s in range(8, 12):
                xs[s] = load_x(s)
            load_w(3)
            load_w(4)
            for s in range(12, 16):
                xs[s] = load_x(s)
            load_w(5)
            for s in range(16, 20):
                xs[s] = load_x(s)
            load_w(6)
            load_w(7)

            rv_of = {}

            def load_rv(s0s):
                # each 8-reg batch lands in alternating ring halves; the
                # in-order tensor queue makes reuse safe once the prior
                # group's matmuls have been emitted
                for s0 in s0s:
                    rvs = load_widx(s0)
                    for j in range(4):
                        rv_of[s0 + j] = (rvs[2 * j], rvs[2 * j + 1])

            # --- phase 1: k-outer groups sized to hide the W stream ---
            load_rv((0,))
            kouter_group(range(0, 4), xs, rv_of)
            load_rv((4, 8))
            kouter_group(range(4, 12), xs, rv_of)
            load_rv((12, 16))
            kouter_group(range(12, 20), xs, rv_of)
            for s in range(20):
                del xs[s]

            # --- phase 2: steady state, sample-major ---
            P2 = 20
            for s in range(P2, S):
                xA, xB = load_x(s)
                if s % 4 == 0:
                    rvs = load_widx(s)
                    for j in range(4):
                        if s + j < S:
                            rv_of[s + j] = (rvs[2 * j], rvs[2 * j + 1])
                rvA, rvB = rv_of[s]

                ps = pspool.tile([128, D_MODEL], f32, tag="ps",
                                 name=f"ps2_{s}")
                for k in range(N_KCH):
                    mm(ps, xA, rvA, k, start=(k == 0), stop=False)
                    mm(ps, xB, rvB, k, start=False, stop=(k == N_KCH - 1))
                combine(s, ps)

    nc.finalize()  # Bacc: reg graph-coloring + codegen passes, then freeze
    return nc


def _gates_np(logits, moe_masks):
    """Mirror reference _gates in numpy (fp32)."""
    lg = logits.astype(np.float32)
    m = lg.max(axis=1, keepdims=True)
    e = np.exp(lg - m)
    g = e / e.sum(axis=1, keepdims=True)
    g = g * (moe_masks == 1).astype(np.float32)
    # top-2, ties -> lower index first (matches jax.lax.top_k)
    top_idx = np.argsort(-g, axis=1, kind="stable")[:, :TOP_K]
    rows = np.arange(g.shape[0])[:, None]
    gsel = g[rows, top_idx]                                  # [B, 2]
    gsel = gsel / (gsel.sum(axis=1, keepdims=True) + EPS)
    return gsel.astype(np.float32), top_idx.astype(np.int32)


def _pack_x(xs):
    """[B, L, K_PAD] f32 -> [B, 128, sub, L] bf16 with k = sub*128 + p."""
    return np.ascontiguousarray(
        xs.astype(BF16).reshape(B, L, N_KCH, 128).transpose(0, 3, 2, 1))


def _prep_inputs(cycle_curve_data, logits, moe_masks, W, b):
    gsel, top_idx = _gates_np(logits, moe_masks)

    xf = cycle_curve_data.reshape(B, L, FEAT).astype(np.float32, copy=False)
    xq = np.zeros((B, L, K_PAD), np.float32)
    xq[:, :, :FEAT] = xf
    xq[:, :, FEAT] = 1.0
    xa = _pack_x(xq * gsel[:, 0].reshape(B, 1, 1))
    xb = _pack_x(xq * gsel[:, 1].reshape(B, 1, 1))

    w_aug = np.zeros((NUM_EXPERTS, K_PAD, D_MODEL), np.float32)
    w_aug[:, :FEAT, :] = W.astype(np.float32)
    w_aug[:, FEAT, :] = b.astype(np.float32)
    # [E, k, p, 512] -> [k, p, E, 512]
    w_host = np.ascontiguousarray(
        w_aug.astype(BF16).reshape(NUM_EXPERTS, N_KCH, 128, D_MODEL)
        .transpose(1, 2, 0, 3)).reshape(N_KCH, 128, NUM_EXPERTS * D_MODEL)

    in_maps = []
    for c in range(N_CORES):
        sl = slice(c * S, (c + 1) * S)
        widx = (top_idx[sl].reshape(1, 2 * S) * D_MODEL).astype(np.int32)
        in_maps.append({
            "xa": xa[sl],
            "xb": xb[sl],
            "w": w_host,
            "widx": widx,
        })
    return in_maps


def kernel(cycle_curve_data, logits, moe_masks, W, b):
    if "nc" not in _CACHE:
        _CACHE["nc"] = _build_nc()
    nc = _CACHE["nc"]

    in_maps = _prep_inputs(cycle_curve_data, logits, moe_masks, W, b)

    trace = bool(int(os.environ.get("KERNEL_PROFILE", "0")))
    res = run_bass_kernel_spmd(
        nc, in_maps, core_ids=list(range(N_CORES)), trace=trace
    )
    _CACHE["last_results"] = res

    out = np.empty((B, L, D_MODEL), ml_dtypes.bfloat16)
    for c in range(N_CORES):
        out[c * S: (c + 1) * S] = res.results[c]["y"]
    return out
